# revision 3
# baseline (speedup 1.0000x reference)
"""GNN message-passing kernel for 8 Trainium2 NeuronCores.

Strategy:
  - Host: sort edges by receiver, partition nodes into 128-node windows,
    assign contiguous window ranges to cores (all edges of a node land on
    one core -> no collectives). Pre-gather V rows for senders/receivers
    into a feature-major [384, cols] edge-input per core.
  - Device: dense edge MLP (feature-major), segment-sum via one-hot
    matmul accumulated in PSUM per 128-node window, mean-scale, PE
    transpose, fused node MLP. Everything fp32.
"""

import os
import numpy as np

B, N, M = 1, 100000, 800000
D, HID = 128, 256
NCORES = 8
WIN = 128  # nodes per window
NW_GLOBAL = 784  # windows globally (100352 padded nodes)
NW = NW_GLOBAL // NCORES  # 98 windows per core
NPAD = NW_GLOBAL * WIN  # 100352
PAD_REL = 200.0  # one-hot rel index for padding slots (matches nothing)

_CACHE = {}

# Set by kernel() after a traced run (GNN_TRACE=1): exec_time_ns etc.
LAST_INFO = {}


def _chunks(total):
    """Split `total` columns into pieces of <=512."""
    out = []
    off = 0
    while off < total:
        sz = min(512, total - off)
        out.append((off, sz))
        off += sz
    return out


def _build_program(t_fix):
    import concourse.bacc as bacc
    import concourse.bass as bass
    import concourse.tile as tile
    from concourse import mybir
    from contextlib import ExitStack

    f32 = mybir.dt.float32
    Gelu = mybir.ActivationFunctionType.Gelu
    Copy = mybir.ActivationFunctionType.Copy
    is_eq = mybir.AluOpType.is_equal
    mult = mybir.AluOpType.mult

    cols = NW * t_fix * 128  # padded edge slots per core
    nrows = NW * WIN  # nodes per core (padded)

    nc = bacc.Bacc(
        "TRN2",
        target_bir_lowering=False,
        debug=False,
        enable_asserts=False,
        num_devices=NCORES,
    )

    edge_in = nc.dram_tensor("edge_in", [3 * D, cols], f32, kind="ExternalInput")
    relT = nc.dram_tensor("relT", [128, NW * t_fix], f32, kind="ExternalInput")
    vT = nc.dram_tensor("vT", [D, nrows], f32, kind="ExternalInput")
    icnt = nc.dram_tensor("icnt", [WIN, NW], f32, kind="ExternalInput")
    we1 = nc.dram_tensor("we1", [D, 3 * HID], f32, kind="ExternalInput")
    we2 = nc.dram_tensor("we2", [D, 2 * D], f32, kind="ExternalInput")
    wn1 = nc.dram_tensor("wn1", [D, 2 * HID], f32, kind="ExternalInput")
    wn2 = nc.dram_tensor("wn2", [D, 2 * D], f32, kind="ExternalInput")
    be1 = nc.dram_tensor("be1", [D, 2], f32, kind="ExternalInput")
    bn1 = nc.dram_tensor("bn1", [D, 2], f32, kind="ExternalInput")
    be2b = nc.dram_tensor("be2b", [D, D], f32, kind="ExternalInput")
    bn2b = nc.dram_tensor("bn2b", [D, D], f32, kind="ExternalInput")
    iota = nc.dram_tensor("iota", [128, 128], f32, kind="ExternalInput")
    ident = nc.dram_tensor("ident", [128, 128], f32, kind="ExternalInput")
    emb_out = nc.dram_tensor("emb_out", [cols, D], f32, kind="ExternalOutput")
    node_out = nc.dram_tensor("node_out", [nrows, D], f32, kind="ExternalOutput")

    chunks = _chunks(t_fix * 128)

    with tile.TileContext(nc) as tc, ExitStack() as ctx:
        const = ctx.enter_context(tc.tile_pool(name="const", bufs=1))

        def load_const(ap, shape):
            t = const.tile(shape, f32, tag=ap.name)
            nc.sync.dma_start(t[:], ap.ap()[:])
            return t

        we1_t = load_const(we1, [D, 3 * HID])
        we2_t = load_const(we2, [D, 2 * D])
        wn1_t = load_const(wn1, [D, 2 * HID])
        wn2_t = load_const(wn2, [D, 2 * D])
        be1_t = load_const(be1, [D, 2])
        bn1_t = load_const(bn1, [D, 2])
        be2b_t = load_const(be2b, [D, D])
        bn2b_t = load_const(bn2b, [D, D])
        iota_t = load_const(iota, [128, 128])
        ident_t = load_const(ident, [128, 128])
        rel_t = load_const(relT, [128, NW * t_fix])
        icnt_t = load_const(icnt, [WIN, NW])

        inp = ctx.enter_context(tc.tile_pool(name="inp", bufs=3))
        hidp = ctx.enter_context(tc.tile_pool(name="hid", bufs=3))
        embp = ctx.enter_context(tc.tile_pool(name="emb", bufs=6))
        ohp = ctx.enter_context(tc.tile_pool(name="ohp", bufs=4))
        vtp = ctx.enter_context(tc.tile_pool(name="vtp", bufs=2))
        mnp = ctx.enter_context(tc.tile_pool(name="mnp", bufs=2))
        outp = ctx.enter_context(tc.tile_pool(name="outp", bufs=2))
        ps_l1 = ctx.enter_context(tc.tile_pool(name="ps_l1", bufs=3, space="PSUM"))
        ps_l2 = ctx.enter_context(tc.tile_pool(name="ps_l2", bufs=2, space="PSUM"))
        ps_seg = ctx.enter_context(tc.tile_pool(name="ps_seg", bufs=2, space="PSUM"))
        ps_msc = ctx.enter_context(tc.tile_pool(name="ps_msc", bufs=1, space="PSUM"))

        for w in range(NW):
            col0 = w * t_fix * 128
            seg = ps_seg.tile([128, 128], f32, tag="seg")
            tglob = 0
            for coff, csz in chunks:
                ins_k = []
                for k in range(3):
                    it = inp.tile([128, 512], f32, tag=f"in{k}")
                    nc.sync.dma_start(
                        it[:, :csz],
                        edge_in.ap()[k * 128 : (k + 1) * 128,
                                     col0 + coff : col0 + coff + csz],
                    )
                    ins_k.append(it)
                hid = []
                for m in range(2):
                    ps = ps_l1.tile([128, 512], f32, tag="l1")
                    for k in range(3):
                        nc.tensor.matmul(
                            ps[:, :csz],
                            we1_t[:, k * 256 + m * 128 : k * 256 + (m + 1) * 128],
                            ins_k[k][:, :csz],
                            start=(k == 0),
                            stop=(k == 2),
                        )
                    h = hidp.tile([128, 512], f32, tag=f"h{m}")
                    nc.scalar.activation(
                        h[:, :csz], ps[:, :csz], Gelu, bias=be1_t[:, m : m + 1]
                    )
                    hid.append(h)
                for s in range(csz // 128):
                    ps2 = ps_l2.tile([128, 128], f32, tag="l2")
                    for m in range(2):
                        nc.tensor.matmul(
                            ps2[:],
                            hid[m][:, s * 128 : (s + 1) * 128],
                            we2_t[:, m * 128 : (m + 1) * 128],
                            start=(m == 0),
                            stop=(m == 1),
                        )
                    emb = embp.tile([128, 128], f32, tag="emb")
                    nc.vector.tensor_add(emb[:], ps2[:], be2b_t[:])
                    g = w * t_fix + tglob
                    nc.sync.dma_start(
                        emb_out.ap()[g * 128 : (g + 1) * 128, :], emb[:]
                    )
                    oh = ohp.tile([128, 128], f32, tag="oh")
                    nc.vector.tensor_scalar(
                        oh[:], iota_t[:], rel_t[:, g : g + 1], None, is_eq
                    )
                    nc.tensor.matmul(
                        seg[:],
                        oh[:],
                        emb[:],
                        start=(tglob == 0),
                        stop=(tglob == t_fix - 1),
                        skip_group_check=True,
                    )
                    tglob += 1

            mean = mnp.tile([128, 128], f32, tag="mean")
            nc.vector.tensor_scalar(mean[:], seg[:], icnt_t[:, w : w + 1], None, mult)
            pmt = ps_msc.tile([128, 128], f32, tag="msc")
            nc.tensor.transpose(pmt[:], mean[:], ident_t[:])
            meanT = mnp.tile([128, 128], f32, tag="meanT")
            nc.scalar.activation(meanT[:], pmt[:], Copy)
            vt_t = vtp.tile([128, 128], f32, tag="vt")
            nc.sync.dma_start(vt_t[:], vT.ap()[:, w * 128 : (w + 1) * 128])
            nhid = []
            for m in range(2):
                ps = ps_l1.tile([128, 512], f32, tag="l1")
                for k in range(2):
                    rhs = vt_t if k == 0 else meanT
                    nc.tensor.matmul(
                        ps[:, :128],
                        wn1_t[:, k * 256 + m * 128 : k * 256 + (m + 1) * 128],
                        rhs[:],
                        start=(k == 0),
                        stop=(k == 1),
                    )
                h = hidp.tile([128, 512], f32, tag=f"h{m}")
                nc.scalar.activation(
                    h[:, :128], ps[:, :128], Gelu, bias=bn1_t[:, m : m + 1]
                )
                nhid.append(h)
            ps2 = ps_l2.tile([128, 128], f32, tag="l2")
            for m in range(2):
                nc.tensor.matmul(
                    ps2[:],
                    nhid[m][:, :128],
                    wn2_t[:, m * 128 : (m + 1) * 128],
                    start=(m == 0),
                    stop=(m == 1),
                )
            nout = outp.tile([128, 128], f32, tag="no")
            nc.vector.tensor_add(nout[:], ps2[:], bn2b_t[:])
            nc.sync.dma_start(node_out.ap()[w * 128 : (w + 1) * 128, :], nout[:])

    nc.compile()
    return nc


def _install_trace_shims():
    """NTFF tracing under axon needs antenv.axon_hooks (absent on this image)
    and an S3 upload that can't run here. Shim both; dev-only (GNN_TRACE=1)."""
    import sys
    import types

    try:
        from antenv.axon_hooks import get_axon_ntff_profile_hook  # noqa: F401
    except ImportError:
        import antenv
        from trn_agent_boot.trn_boot import _ntff_profile_via_ctypes

        mod = types.ModuleType("antenv.axon_hooks")
        _hook = [_ntff_profile_via_ctypes("/opt/axon/libaxon_pjrt.so")]
        mod.get_axon_ntff_profile_hook = lambda: _hook[0]
        mod.set_axon_ntff_profile_hook = lambda h: _hook.__setitem__(0, h)
        sys.modules["antenv.axon_hooks"] = mod
        antenv.axon_hooks = mod

    import concourse.bass_utils as bu

    bu.upload_artifacts = lambda tmpdir: tmpdir


def kernel(V, E, edges, We1, be1, We2, be2, Wn1, bn1, Wn2, bn2):
    from concourse.bass_utils import run_bass_kernel_spmd

    V = np.asarray(V, np.float32)
    E = np.asarray(E, np.float32)
    edges = np.asarray(edges)
    We1 = np.asarray(We1, np.float32)
    be1 = np.asarray(be1, np.float32)
    We2 = np.asarray(We2, np.float32)
    be2 = np.asarray(be2, np.float32)
    Wn1 = np.asarray(Wn1, np.float32)
    bn1 = np.asarray(bn1, np.float32)
    Wn2 = np.asarray(Wn2, np.float32)
    bn2 = np.asarray(bn2, np.float32)

    V0, E0 = V[0], E[0]
    send = np.asarray(edges[0, :, 0], np.int64)
    recv = np.asarray(edges[0, :, 1], np.int64)

    # ---- host preprocessing ----
    perm = np.argsort(recv, kind="stable")
    send_s = send[perm]
    recv_s = recv[perm]

    cnt = np.bincount(recv, minlength=NPAD).astype(np.float32)
    inv_cnt = 1.0 / np.maximum(cnt, 1.0)

    wstart = np.searchsorted(recv_s, np.arange(0, NPAD + 1, WIN))  # [785]
    wcnt = np.diff(wstart)  # edges per window [784]
    t_fix = max(1, int(np.ceil(wcnt.max() / 128)))
    cols = NW * t_fix * 128
    wtilecols = t_fix * 128

    VT = np.ascontiguousarray(V0.T)  # [128, N]
    VTp = np.zeros((D, NPAD), np.float32)
    VTp[:, :N] = VT
    ET = np.ascontiguousarray(E0.T)  # [128, M]

    # weight repacking (lhsT layouts)
    we1_in = We1.reshape(3, 128, 2 * D).transpose(1, 0, 2).reshape(128, 3 * HID)
    we2_in = We2.reshape(2, 128, D).transpose(1, 0, 2).reshape(128, 2 * D)
    wn1_in = Wn1.reshape(2, 128, HID).transpose(1, 0, 2).reshape(128, 2 * HID)
    wn2_in = Wn2.reshape(2, 128, D).transpose(1, 0, 2).reshape(128, 2 * D)
    be1_in = np.ascontiguousarray(be1.reshape(2, 128).T)
    bn1_in = np.ascontiguousarray(bn1.reshape(2, 128).T)
    be2b = np.tile(be2[None, :], (128, 1)).astype(np.float32)
    bn2b = np.tile(bn2[None, :], (128, 1)).astype(np.float32)
    iota_in = np.tile(np.arange(128, dtype=np.float32)[None, :], (128, 1))
    ident_in = np.eye(128, dtype=np.float32)

    in_maps = []
    g_all, valid_all = [], []
    for c in range(NCORES):
        g = np.zeros(cols, np.int64)
        valid = np.zeros(cols, bool)
        for wl in range(NW):
            wg = c * NW + wl
            n_e = wcnt[wg]
            s0 = wl * wtilecols
            g[s0 : s0 + n_e] = np.arange(wstart[wg], wstart[wg] + n_e)
            valid[s0 : s0 + n_e] = True
        g_all.append(g)
        valid_all.append(valid)

        win_of_slot = (c * NW + np.arange(cols) // wtilecols) * WIN
        rel = recv_s[g].astype(np.float32) - win_of_slot.astype(np.float32)
        rel[~valid] = PAD_REL
        relT_in = np.ascontiguousarray(rel.reshape(NW * t_fix, 128).T)

        edge_in = np.empty((3 * D, cols), np.float32)
        edge_in[0:D] = VT[:, send_s[g]]
        edge_in[D : 2 * D] = VT[:, recv_s[g]]
        edge_in[2 * D : 3 * D] = ET[:, perm[g]]

        nb = c * NW * WIN
        vt_in = np.ascontiguousarray(VTp[:, nb : nb + NW * WIN])
        icnt_in = np.ascontiguousarray(
            inv_cnt[nb : nb + NW * WIN].reshape(NW, WIN).T
        )

        in_maps.append(
            dict(
                edge_in=edge_in,
                relT=relT_in,
                vT=vt_in,
                icnt=icnt_in,
                we1=we1_in,
                we2=we2_in,
                wn1=wn1_in,
                wn2=wn2_in,
                be1=be1_in,
                bn1=bn1_in,
                be2b=be2b,
                bn2b=bn2b,
                iota=iota_in,
                ident=ident_in,
            )
        )

    key = t_fix
    if key not in _CACHE:
        _CACHE[key] = _build_program(t_fix)
    nc = _CACHE[key]

    trace = os.environ.get("GNN_TRACE", "0") == "1"
    tmpdir = None
    if trace:
        tmpdir = os.environ.get("GNN_TRACE_DIR", "/tmp/gnn_trace")
        os.makedirs(tmpdir, exist_ok=True)
        _install_trace_shims()
    res = run_bass_kernel_spmd(
        nc, in_maps, list(range(NCORES)), trace=trace, tmpdir=tmpdir
    )
    LAST_INFO.clear()
    LAST_INFO.update(
        exec_time_ns=res.exec_time_ns,
        mean_exec_time_ns=res.mean_exec_time_ns,
        trace=res.instructions_and_trace[1] if res.instructions_and_trace else None,
    )

    # ---- assembly ----
    node_emb = np.empty((NPAD, D), np.float32)
    edge_emb_s = np.empty((M, D), np.float32)
    for c in range(NCORES):
        out = res.results[c]
        nb = c * NW * WIN
        node_emb[nb : nb + NW * WIN] = out["node_out"]
        edge_emb_s[g_all[c][valid_all[c]]] = out["emb_out"][valid_all[c]]

    edge_emb = np.empty((M, D), np.float32)
    edge_emb[perm] = edge_emb_s
    return node_emb[None, :N, :], edge_emb[None, :, :]
